# revision 1
# baseline (speedup 1.0000x reference)
"""ContextQueryAttention (BiDAF-style) Trainium2 Bass kernel.

Full inputs -> full output; internally data-parallel over batch across 8
NeuronCores (4 batches per core).

Per-batch math (b dropped; C:[d,t], Q:[d,j], d=512, t=1024, j=128):
  H = C^T, U = Q^T
  S[t,j]  = hbias[t] + ubias[j] + sum_d w_hu[d]*C[d,t]*Q[d,j]
  S_q     = softmax_j(mask(S, mask_Q))         # masked -> -1e30
  S_c     = softmax_t(mask(S, mask_C))
  A       = S_q @ U                            # (t,d)
  q2c     = S_c^T @ H                          # (j,d)
  Bmat    = S_q @ q2c                          # (t,d)
  out     = [H; A; H*A; H*Bmat] as (4d, t)

Layout strategy: everything "feature-on-partitions":
  S^T [j,t] accumulated on PE (fp32) incl. hbias via broadcast-w_h weights;
  ubias + mask_Q applied per-partition (j) on ACT during PSUM evacuation;
  mask_C additively via rank-1 fp32r matmuls (S - 1e30 == -1e30 exactly in f32);
  softmax_j via PE transpose -> rowmax -> ACT exp(accum_out) -> normalize ->
  transpose back; softmax_t free-dim in ^T layout with constant max 100.0;
  A^T / q2c / Bmat^T matmuls in float32r (1 cyc/row);
  output written as [C; A^T; C*A^T; C*B^T] (natural d-on-partition layout).
"""

import numpy as np

import concourse.bass as bass
import concourse.tile as tile
from concourse import bacc, mybir
from concourse import bass_utils
from concourse.masks import make_identity

F32 = mybir.dt.float32
F32R = mybir.dt.float32r
I32 = mybir.dt.int32

B, T, J, D = 32, 1024, 128, 512
NCORES = 8
BPC = B // NCORES  # batches per core
NEG = -1.0e30
MHAT = 100.0  # fixed max-subtraction constant for the softmax over t
NCH = D // 128  # 4 d-chunks
NTC = T // 128  # 8 t-chunks

DEBUG = False

# Experiment switches
S_CORE_F32R = False  # S/hbias core matmuls in fp32r instead of fp32


def _emit_batch(nc, tc, pools, consts, aps, b):
    """Emit instructions for one batch."""
    (cin, qin, mid, outp, psS_pool, tp_pool, mm_pool, sm) = pools
    (identity, ones128, w_col, W_H, ones_r32, r2, mqf_all, mhat_neg) = consts
    (C, Q, out) = aps

    # ---- loads ----
    Ct = []
    for i in range(NCH):
        t = cin.tile([128, T], F32, tag="ct")
        nc.sync.dma_start(t[:], C[b, 128 * i:128 * (i + 1), :])
        Ct.append(t)
    Qt = []
    for i in range(NCH):
        t = qin.tile([128, J], F32, tag="qt")
        nc.sync.dma_start(t[:], Q[b, 128 * i:128 * (i + 1), :])
        Qt.append(t)

    # ---- Qw = Q * w_hu (per-partition scalar), fp32 or fp32r ----
    qw_dt = F32R if S_CORE_F32R else F32
    Qw = []
    for i in range(NCH):
        t = sm.tile([128, J], qw_dt, tag="qw", bufs=8, name=f"qw{b}_{i}")
        nc.vector.tensor_scalar_mul(t[:], Qt[i][:], w_col[:, 8 + i:9 + i])
        Qw.append(t)

    # ---- ubias[j] = sum_d w_u[d] Q[d,j] : N=1 matmuls -> [j,1] ----
    ub_ps = mm_pool.tile([128, 8], F32, tag="mm")
    for i in range(NCH):
        nc.tensor.matmul(ub_ps[:, 0:1], Qt[i][:], w_col[:, 4 + i:5 + i],
                         start=(i == 0), stop=(i == NCH - 1))

    # per-batch mask_Q columns: scale = mqf, bias = mqf*ub + (mqf-1)*1e30
    mqf = mqf_all[:, b:b + 1]
    ub = sm.tile([128, 1], F32, tag="ub")
    nc.vector.tensor_copy(ub[:], ub_ps[:, 0:1])
    addc = sm.tile([128, 1], F32, tag="addc")
    # addc = mqf*1e30 - 1e30
    nc.vector.tensor_scalar(addc[:], mqf, 1.0e30, -1.0e30,
                            op0=mybir.AluOpType.mult, op1=mybir.AluOpType.add)
    mub = sm.tile([128, 1], F32, tag="mub")
    nc.vector.tensor_mul(mub[:], mqf, ub[:])
    nc.vector.tensor_add(addc[:], addc[:], mub[:])

    # ---- S^T core + hbias into PSUM: one bank per t-half (bufs=3 lets
    # the next batch start accumulating while this batch's Sc-exp reads) ----
    r2row = sm.tile([1, T], F32R, tag="r2row", bufs=2)
    nc.sync.dma_start(r2row[:], r2[b:b + 1, :])
    Smq = mid.tile([128, T], F32, tag="smq")
    ecT = mid.tile([128, T], F32, tag="ect")
    csum2 = sm.tile([128, 2], F32, tag="csum2")
    for h in range(2):
        sl = slice(512 * h, 512 * (h + 1))
        psSh = psS_pool.tile([128, 512], F32, tag="psS", name=f"psS{b}_{h}")
        for i in range(NCH):
            nc.tensor.matmul(psSh[:], Qw[i][:], Ct[i][:, sl],
                             start=(i == 0), stop=False)
        for i in range(NCH):
            nc.tensor.matmul(psSh[:], W_H[i][:], Ct[i][:, sl],
                             start=False, stop=False)
        # Smq half = psS*mqf + addc  (ACT, PSUM -> SBUF)
        nc.scalar.activation(Smq[:, sl], psSh[:],
                             mybir.ActivationFunctionType.Identity,
                             bias=addc[:], scale=mqf)
        # rank-1 mask_C (+hbias already in): psS += ones_j x r2[t]
        nc.tensor.matmul(psSh[:], ones_r32[:], r2row[:, sl],
                         start=False, stop=True)
        # S_c path: exp with constant max, fused half-sum
        nc.scalar.activation(ecT[:, sl], psSh[:],
                             mybir.ActivationFunctionType.Exp,
                             bias=mhat_neg[:], scale=1.0,
                             accum_out=csum2[:, h:h + 1])
    csum = sm.tile([128, 1], F32, tag="csum")
    nc.vector.tensor_add(csum[:], csum2[:, 0:1], csum2[:, 1:2])
    rc = sm.tile([128, 1], F32, tag="rc")
    nc.vector.reciprocal(rc[:], csum[:])

    # ---- S_q path: transpose Smq -> [t,j] tiles (packed 4 per PSUM bank) ----
    tpq = [tp_pool.tile([128, 512], F32, tag="tp", name=f"tpq{b}_{k}")
           for k in range(2)]
    for c in range(NTC):
        nc.tensor.transpose(tpq[c // 4][:, 128 * (c % 4):128 * (c % 4 + 1)],
                            Smq[:, 128 * c:128 * (c + 1)], identity[:])
    # negated row-max per chunk
    nrmax = sm.tile([128, 8], F32, tag="nrmax")
    for k in range(2):
        nc.vector.tensor_reduce(nrmax[:, 4 * k:4 * (k + 1)],
                                tpq[k][:].rearrange("p (c f) -> p c f", f=128),
                                op=mybir.AluOpType.max,
                                axis=mybir.AxisListType.X, negate=True)
    # exp with per-row bias, fused row-sums
    e_sb = mid.tile([128, T], F32, tag="esb")
    esum = sm.tile([128, 8], F32, tag="esum")
    for c in range(NTC):
        nc.scalar.activation(e_sb[:, 128 * c:128 * (c + 1)],
                             tpq[c // 4][:, 128 * (c % 4):128 * (c % 4 + 1)],
                             mybir.ActivationFunctionType.Exp,
                             bias=nrmax[:, c:c + 1],
                             accum_out=esum[:, c:c + 1])
    resum = sm.tile([128, 8], F32, tag="resum")
    nc.vector.reciprocal(resum[:], esum[:])
    # normalize
    en = mid.tile([128, T], F32, tag="en")
    for c in range(NTC):
        nc.vector.tensor_scalar_mul(en[:, 128 * c:128 * (c + 1)],
                                    e_sb[:, 128 * c:128 * (c + 1)],
                                    resum[:, c:c + 1])
    # transpose back -> S_q^T [j,t] fp32r
    tb = [tp_pool.tile([128, 512], F32, tag="tp", name=f"tb{b}_{k}")
          for k in range(2)]
    for c in range(NTC):
        nc.tensor.transpose(tb[c // 4][:, 128 * (c % 4):128 * (c % 4 + 1)],
                            en[:, 128 * c:128 * (c + 1)], identity[:])
    SqT = mid.tile([128, T], F32R, tag="sqt")
    for k in range(2):
        nc.any.tensor_copy(SqT[:, 512 * k:512 * (k + 1)], tb[k][:])

    # ---- e_c transposes -> [t,j] fp32r chunks ----
    te = [tp_pool.tile([128, 512], F32, tag="tp", name=f"te{b}_{k}")
          for k in range(2)]
    for c in range(NTC):
        nc.tensor.transpose(te[c // 4][:, 128 * (c % 4):128 * (c % 4 + 1)],
                            ecT[:, 128 * c:128 * (c + 1)], identity[:])
    ec_sb = mid.tile([128, T], F32R, tag="ecsb")
    for k in range(2):
        nc.any.tensor_copy(ec_sb[:, 512 * k:512 * (k + 1)], te[k][:])

    # ---- H = C^T tiles [t,d] fp32r (4 transposes per t-chunk) ----
    H_sb = []
    for c in range(NTC):
        tH = tp_pool.tile([128, 512], F32, tag="tp")
        for i in range(NCH):
            nc.tensor.transpose(tH[:, 128 * i:128 * (i + 1)],
                                Ct[i][:, 128 * c:128 * (c + 1)], identity[:])
        hs = mid.tile([128, 512], F32R, tag="hsb", bufs=10, name=f"hsb{b}_{c}")
        nc.any.tensor_copy(hs[:], tH[:])
        H_sb.append(hs)

    # ---- Q^T [j,d] fp32r ----
    tQ = tp_pool.tile([128, 512], F32, tag="tp")
    for i in range(NCH):
        nc.tensor.transpose(tQ[:, 128 * i:128 * (i + 1)], Qt[i][:], identity[:])
    QT = mid.tile([128, 512], F32R, tag="qT")
    nc.any.tensor_copy(QT[:], tQ[:])

    # ---- q2c = (1/csum) * sum_c e_c[c].T @ H[c]  -> [j,d] fp32r ----
    psq = mm_pool.tile([128, 512], F32, tag="mm")
    for c in range(NTC):
        nc.tensor.matmul(psq[:], ec_sb[:, 128 * c:128 * (c + 1)], H_sb[c][:],
                         start=(c == 0), stop=(c == NTC - 1))
    q2c = mid.tile([128, 512], F32R, tag="q2c")
    nc.vector.tensor_scalar_mul(q2c[:], psq[:], rc[:])

    # ---- A^T, Bmat^T (fp32r), outputs ----
    A_sb = []
    for m in range(NCH):
        a = outp.tile([128, T], F32, tag="asb")
        for h in range(2):
            sl = slice(512 * h, 512 * (h + 1))
            psA = mm_pool.tile([128, 512], F32, tag="mm")
            nc.tensor.matmul(psA[:], QT[:, 128 * m:128 * (m + 1)], SqT[:, sl],
                             start=True, stop=True)
            nc.any.tensor_copy(a[:, sl], psA[:])
        A_sb.append(a)

    HB = []
    for m in range(NCH):
        hb = outp.tile([128, T], F32, tag="hbsb")
        for h in range(2):
            sl = slice(512 * h, 512 * (h + 1))
            psB = mm_pool.tile([128, 512], F32, tag="mm")
            nc.tensor.matmul(psB[:], q2c[:, 128 * m:128 * (m + 1)], SqT[:, sl],
                             start=True, stop=True)
            nc.vector.tensor_mul(hb[:, sl], Ct[m][:, sl], psB[:])
        HB.append(hb)

    HA = []
    for m in range(NCH):
        ha = outp.tile([128, T], F32, tag="hasb")
        nc.gpsimd.tensor_mul(ha[:], Ct[m][:], A_sb[m][:])
        HA.append(ha)

    # ---- output DMAs: rows [C; A^T; H*A; H*B] ----
    if b == 0 and getattr(nc, "_dbg", None):
        dbg = nc._dbg
        nc.sync.dma_start(dbg["smq"], Smq[:])
        nc.sync.dma_start(dbg["sqt"], SqT[:].bitcast(F32))
        nc.sync.dma_start(dbg["ect"], ecT[:])
        nc.sync.dma_start(dbg["q2c"], q2c[:].bitcast(F32))
        nc.sync.dma_start(dbg["csum"], csum[:])
        nc.sync.dma_start(dbg["ub"], ub[:])
        nc.sync.dma_start(dbg["esb"], e_sb[:])
        nc.sync.dma_start(dbg["nrmax"], nrmax[:])
    for m in range(NCH):
        nc.sync.dma_start(out[b, 128 * m:128 * (m + 1), :], Ct[m][:])
        nc.sync.dma_start(out[b, D + 128 * m:D + 128 * (m + 1), :], A_sb[m][:])
        nc.sync.dma_start(out[b, 2 * D + 128 * m:2 * D + 128 * (m + 1), :],
                          HA[m][:])
        nc.sync.dma_start(out[b, 3 * D + 128 * m:3 * D + 128 * (m + 1), :],
                          HB[m][:])


def _build():
    nc = bacc.Bacc("TRN2", target_bir_lowering=False, debug=False,
                   num_devices=NCORES)
    C = nc.dram_tensor("C", [BPC, D, T], F32, kind="ExternalInput").ap()
    Q = nc.dram_tensor("Q", [BPC, D, J], F32, kind="ExternalInput").ap()
    mask_C = nc.dram_tensor("mask_C", [BPC, T], I32, kind="ExternalInput").ap()
    mask_Q = nc.dram_tensor("mask_Q", [BPC, J], I32, kind="ExternalInput").ap()
    weight = nc.dram_tensor("weight", [3 * D], F32, kind="ExternalInput").ap()
    out = nc.dram_tensor("out", [BPC, 4 * D, T], F32,
                         kind="ExternalOutput").ap()
    dbg = {}
    if DEBUG:
        for nm, shp in [("smq", [128, T]), ("sqt", [128, T]),
                        ("ect", [128, T]), ("q2c", [128, D]),
                        ("csum", [128, 1]), ("ub", [128, 1]),
                        ("psS", [128, T]), ("esb", [128, T]),
                        ("nrmax", [128, 8])]:
            dbg[nm] = nc.dram_tensor(f"dbg_{nm}", shp, F32,
                                     kind="ExternalOutput").ap()

    with tile.TileContext(nc) as tc:
        import contextlib
        with contextlib.ExitStack() as ctx:
            const = ctx.enter_context(tc.tile_pool(name="const", bufs=1))
            cin = ctx.enter_context(tc.tile_pool(name="cin", bufs=6))
            qin = ctx.enter_context(tc.tile_pool(name="qin", bufs=8))
            mid = ctx.enter_context(tc.tile_pool(name="mid", bufs=2))
            outp = ctx.enter_context(tc.tile_pool(name="outp", bufs=5))
            sm = ctx.enter_context(tc.tile_pool(name="sm", bufs=4))
            psS_pool = ctx.enter_context(
                tc.tile_pool(name="psS", bufs=3, space="PSUM"))
            tp_pool = ctx.enter_context(
                tc.tile_pool(name="tp", bufs=3, space="PSUM"))
            mm_pool = ctx.enter_context(
                tc.tile_pool(name="mm", bufs=2, space="PSUM"))

            # ---- constants ----
            identity = const.tile([128, 128], F32, tag="identity")
            make_identity(nc, identity[:])
            ones128 = const.tile([128, 128], F32, tag="ones128")
            nc.gpsimd.memset(ones128[:], 1.0)
            ones_r32 = const.tile([1, 128], F32R, tag="ones_r32")
            nc.vector.tensor_copy(ones_r32[:], ones128[0:1, :])
            # weight -> [128, 12]: cols g*4+c hold weight[g*512 + c*128 + p]
            w_col = const.tile([128, 12], F32, tag="w_col")
            nc.sync.dma_start(
                w_col[:], weight.rearrange("(g c p) -> p (g c)", p=128, c=4))
            # W_H[i]: w_h chunk broadcast along free dim (rank-1 weights)
            W_H = []
            wh_dt = F32R if S_CORE_F32R else F32
            for i in range(NCH):
                t = const.tile([128, 128], wh_dt, tag=f"W_H{i}")
                nc.vector.tensor_scalar_mul(t[:], ones128[:], w_col[:, i:i + 1])
                W_H.append(t)
            # mask_C -> r2 = (m-1)*1e30 fp32r, all batches [BPC, 1024]
            mcr = const.tile([BPC, T], I32, tag="mcr")
            nc.sync.dma_start(mcr[:], mask_C)
            mcf = const.tile([BPC, T], F32, tag="mcf")
            nc.vector.tensor_copy(mcf[:], mcr[:])
            r2 = const.tile([BPC, T], F32R, tag="r2")
            nc.vector.tensor_scalar(r2[:], mcf[:], 1.0e30, -1.0e30,
                                    op0=mybir.AluOpType.mult,
                                    op1=mybir.AluOpType.add)
            # mask_Q -> [128, BPC] fp32
            mqi = const.tile([128, BPC], I32, tag="mqi")
            nc.sync.dma_start(mqi[:], mask_Q.rearrange("b j -> j b"))
            mqf_all = const.tile([128, BPC], F32, tag="mqf")
            nc.vector.tensor_copy(mqf_all[:], mqi[:])
            mhat_neg = const.tile([128, 1], F32, tag="mhat")
            nc.gpsimd.memset(mhat_neg[:], -MHAT)

            consts = (identity, ones128, w_col, W_H, ones_r32, r2, mqf_all, mhat_neg)
            nc._dbg = dbg
            pools = (cin, qin, mid, outp, psS_pool, tp_pool, mm_pool, sm)
            for b in range(BPC):
                _emit_batch(nc, tc, pools, consts, (C, Q, out), b)

    nc.compile()
    return nc


_NC_CACHE = None


def _get_nc():
    global _NC_CACHE
    if _NC_CACHE is None:
        _NC_CACHE = _build()
    return _NC_CACHE


def kernel(C, Q, mask_C, mask_Q, weight):
    nc = _get_nc()
    C = np.ascontiguousarray(C, dtype=np.float32)
    Q = np.ascontiguousarray(Q, dtype=np.float32)
    mask_C = np.ascontiguousarray(mask_C, dtype=np.int32)
    mask_Q = np.ascontiguousarray(mask_Q, dtype=np.int32)
    weight = np.ascontiguousarray(weight, dtype=np.float32)
    in_maps = []
    for c in range(NCORES):
        sl = slice(BPC * c, BPC * (c + 1))
        in_maps.append({
            "C": C[sl], "Q": Q[sl], "mask_C": mask_C[sl],
            "mask_Q": mask_Q[sl], "weight": weight,
        })
    res = bass_utils.run_bass_kernel_spmd(nc, in_maps,
                                          core_ids=list(range(NCORES)))
    return np.concatenate([res.results[c]["out"] for c in range(NCORES)],
                          axis=0)



# revision 11
# speedup vs baseline: 1.3003x; 1.3003x over previous
"""ContextQueryAttention (BiDAF-style) Trainium2 Bass kernel, v2.

Full inputs -> full output; internally data-parallel over batch across 8
NeuronCores (4 batches per core).

Per-batch math (b dropped; C:[d,t], Q:[d,j], d=512, t=1024, j=128):
  H = C^T, U = Q^T
  S[t,j]  = hbias[t] + ubias[j] + sum_d w_hu[d]*C[d,t]*Q[d,j]
  S_q     = softmax_j(mask(S, mask_Q))         # masked -> -1e30
  S_c     = softmax_t(mask(S, mask_C))
  A       = S_q @ U                            # (t,d)
  q2c     = S_c^T @ H                          # (j,d)
  Bmat    = S_q @ q2c                          # (t,d)
  out     = [H; A; H*A; H*Bmat] as (4d, t)

v2 changes vs v1:
  - All PE matmuls in fp32r (1 cyc/row at free>=256) incl. the S core and
    the broadcast-w_h hbias matmuls; transposes fed an fp32r identity.
  - mask_C is applied as a per-partition (t) scalar multiply on the
    [t,j]-layout evacuation of exp(S - MHAT) (replaces the plain copy), and
    the S_c column sums come from 8 tiny ap=1 matmuls against ones. This
    removes the rank-1 mask matmuls, the r2 constants and the per-batch
    r2row DMAs. Relies on saturating (non-inf) exp like v1's MHAT trick.
  - Merged DMAs: one C load [128,(c t)], one Q load, one store per output
    block (H/A/HA/HB) with DRAM-side "(c p) t -> p (c t)" APs.
  - Queue split: loads + H store on SP; A/HA/HB stores on ACT. This stops
    store semaphore-waits from head-of-line blocking next-batch loads.
  - e_sb normalized in place; merged SqT/ecsb/H_sb tiles.
"""

import numpy as np

import concourse.bass as bass
import concourse.tile as tile
from concourse import bacc, mybir
from concourse import bass_utils
from concourse.masks import make_identity

F32 = mybir.dt.float32
F32R = mybir.dt.float32r
I32 = mybir.dt.int32

B, T, J, D = 32, 1024, 128, 512
NCORES = 8
BPC = B // NCORES  # batches per core
MHAT = 100.0  # fixed max-subtraction constant for the softmax over t
NCH = D // 128  # 4 d-chunks
NTC = T // 128  # 8 t-chunks


def _emit_batch(nc, tc, pools, consts, aps, b):
    """Emit instructions for one batch."""
    (cin, qin, mid, outp, psS_pool, tp_pool, mm_pool, sm) = pools
    (identity, ones128, onescol, w_col, W_H, mqf_all, mcolf, mhat_neg) = consts
    (C, Q, out) = aps

    # ---- loads (SP queue) + early H store ----
    ct = cin.tile([128, NCH * T], F32, tag="ct")  # chunk i at [:, T*i:T*(i+1)]
    nc.sync.dma_start(ct[:].rearrange("p (c t) -> p c t", c=NCH),
                      C[b, :, :].rearrange("(c p) t -> p c t", c=NCH))
    qt = qin.tile([128, NCH * J], F32, tag="qt")  # chunk i at [:, J*i:J*(i+1)]
    nc.sync.dma_start(qt[:].rearrange("p (c j) -> p c j", c=NCH),
                      Q[b, :, :].rearrange("(c p) j -> p c j", c=NCH))
    nc.sync.dma_start(
        out[b, 0:D, :].rearrange("(c p) t -> p c t", c=NCH),
        ct[:].rearrange("p (c t) -> p c t", c=NCH))

    # fp32r-rounded copy of C for the S-core matmuls (BIR requires fp32r
    # matmul inputs to be produced rounded; DMA output doesn't qualify)
    ctr = mid.tile([128, NCH * T], F32R, tag="ctr", bufs=1, name=f"ctr{b}")
    nc.vector.tensor_copy(ctr[:, 0:2 * T], ct[:, 0:2 * T])
    nc.gpsimd.tensor_copy(ctr[:, 2 * T:4 * T], ct[:, 2 * T:4 * T])

    def ctc(i, sl=slice(None)):  # rounded C chunk i, free-slice sl
        base = T * i
        lo = sl.start if sl.start is not None else 0
        hi = sl.stop if sl.stop is not None else T
        return ctr[:, base + lo:base + hi]

    # ---- Qw = Q * w_hu (per-partition scalar), fp32r ----
    qw = sm.tile([128, NCH * J], F32R, tag="qw", bufs=2, name=f"qw{b}")
    for i in range(NCH):
        nc.vector.tensor_scalar_mul(qw[:, J * i:J * (i + 1)],
                                    qt[:, J * i:J * (i + 1)],
                                    w_col[:, 8 + i:9 + i])

    # ---- ubias[j] = sum_d w_u[d] Q[d,j] : N=1 matmuls -> [j,1] ----
    ub_ps = mm_pool.tile([128, 8], F32, tag="mm")
    for i in range(NCH):
        nc.tensor.matmul(ub_ps[:, 0:1], qt[:, J * i:J * (i + 1)],
                         w_col[:, 4 + i:5 + i],
                         start=(i == 0), stop=(i == NCH - 1))

    # per-batch mask_Q columns: scale = mqf, bias = mqf*ub + (mqf-1)*1e30
    mqf = mqf_all[:, b:b + 1]
    addc = sm.tile([128, 1], F32, tag="addc")
    nc.vector.tensor_scalar(addc[:], mqf, 1.0e30, -1.0e30,
                            op0=mybir.AluOpType.mult, op1=mybir.AluOpType.add)
    mub = sm.tile([128, 1], F32, tag="mub")
    nc.vector.tensor_mul(mub[:], mqf, ub_ps[:, 0:1])
    nc.vector.tensor_add(addc[:], addc[:], mub[:])

    # ---- S^T core + hbias into PSUM (fp32r): one bank per t-half ----
    Smq = mid.tile([128, T], F32, tag="smq", bufs=2, name=f"smq{b}")
    ecT = mid.tile([128, T], F32, tag="ect", bufs=2, name=f"ect{b}")
    for h in range(2):
        sl = slice(512 * h, 512 * (h + 1))
        psSh = psS_pool.tile([128, 512], F32, tag="psS", name=f"psS{b}_{h}")
        for i in range(NCH):
            nc.tensor.matmul(psSh[:], qw[:, J * i:J * (i + 1)], ctc(i, sl),
                             start=(i == 0), stop=False)
        for i in range(NCH):
            nc.tensor.matmul(psSh[:], W_H[i][:], ctc(i, sl),
                             start=False, stop=(i == NCH - 1))
        # S_q path: Smq half = psS*mqf + addc  (ACT, PSUM -> SBUF)
        nc.scalar.activation(Smq[:, sl], psSh[:],
                             mybir.ActivationFunctionType.Identity,
                             bias=addc[:], scale=mqf)
        # S_c path: exp with constant max subtraction (saturating exp)
        nc.scalar.activation(ecT[:, sl], psSh[:],
                             mybir.ActivationFunctionType.Exp,
                             bias=mhat_neg[:], scale=1.0)

    # ---- S_q path: transpose Smq -> [t,j] tiles (packed 4 per PSUM bank) ----
    tpq = [tp_pool.tile([128, 512], F32, tag="tp", name=f"tpq{b}_{k}")
           for k in range(2)]
    for c in range(NTC):
        nc.tensor.transpose(tpq[c // 4][:, 128 * (c % 4):128 * (c % 4 + 1)],
                            Smq[:, 128 * c:128 * (c + 1)], identity[:])
    # negated row-max per chunk
    nrmax = sm.tile([128, 8], F32, tag="nrmax")
    for k in range(2):
        nc.vector.tensor_reduce(nrmax[:, 4 * k:4 * (k + 1)],
                                tpq[k][:].rearrange("p (c f) -> p c f", f=128),
                                op=mybir.AluOpType.max,
                                axis=mybir.AxisListType.X, negate=True)
    # exp with per-row bias, fused row-sums; then normalize in place
    e_sb = mid.tile([128, T], F32, tag="smq", bufs=2, name=f"esb{b}")
    esum = sm.tile([128, 8], F32, tag="esum")
    for c in range(NTC):
        nc.scalar.activation(e_sb[:, 128 * c:128 * (c + 1)],
                             tpq[c // 4][:, 128 * (c % 4):128 * (c % 4 + 1)],
                             mybir.ActivationFunctionType.Exp,
                             bias=nrmax[:, c:c + 1],
                             accum_out=esum[:, c:c + 1])
    resum = sm.tile([128, 8], F32, tag="resum")
    nc.vector.reciprocal(resum[:], esum[:])
    for c in range(NTC):
        nc.vector.tensor_scalar_mul(e_sb[:, 128 * c:128 * (c + 1)],
                                    e_sb[:, 128 * c:128 * (c + 1)],
                                    resum[:, c:c + 1])
    # transpose back -> S_q^T [j,t] fp32r
    tb = [tp_pool.tile([128, 512], F32, tag="tp", name=f"tb{b}_{k}")
          for k in range(2)]
    for c in range(NTC):
        nc.tensor.transpose(tb[c // 4][:, 128 * (c % 4):128 * (c % 4 + 1)],
                            e_sb[:, 128 * c:128 * (c + 1)], identity[:])
    SqT = mid.tile([128, T], F32R, tag="ect", bufs=2, name=f"sqt{b}")
    for k in range(2):
        nc.scalar.activation(SqT[:, 512 * k:512 * (k + 1)], tb[k][:],
                             mybir.ActivationFunctionType.Identity)

    # ---- e_c transposes -> [t,j] fp32r chunks, mask_C applied per-row ----
    te = [tp_pool.tile([128, 512], F32, tag="tp", name=f"te{b}_{k}")
          for k in range(2)]
    for c in range(NTC):
        nc.tensor.transpose(te[c // 4][:, 128 * (c % 4):128 * (c % 4 + 1)],
                            ecT[:, 128 * c:128 * (c + 1)], identity[:])
    ec_sb = mid.tile([128, T], F32R, tag="ecsb", bufs=2, name=f"ecsb{b}")
    for k in range(2):
        for q in range(4):
            c = 4 * k + q
            nc.vector.tensor_scalar_mul(
                ec_sb[:, 128 * c:128 * (c + 1)],
                te[k][:, 128 * q:128 * (q + 1)],
                mcolf[:, 8 * b + c:8 * b + c + 1])

    # ---- csum[j] = sum_t masked-e_c: tiny ap=1 matmuls against ones ----
    cs_ps = mm_pool.tile([128, 8], F32, tag="mm")
    for c in range(NTC):
        nc.tensor.matmul(cs_ps[:, 0:8], ec_sb[:, 128 * c:128 * (c + 1)],
                         onescol[:],
                         start=(c == 0), stop=(c == NTC - 1))
    rc = sm.tile([128, 1], F32, tag="rc")
    nc.vector.reciprocal(rc[:], cs_ps[:, 0:1])

    # ---- H = C^T tiles [t,d] fp32r (4 transposes per t-chunk) ----
    hsb = mid.tile([128, NTC * 512], F32R, tag="hsb", bufs=1, name=f"hsb{b}")
    for c in range(NTC):
        tH = tp_pool.tile([128, 512], F32, tag="tp", name=f"tH{b}_{c}")
        for i in range(NCH):
            nc.tensor.transpose(tH[:, 128 * i:128 * (i + 1)],
                                ct[:, T * i + 128 * c:T * i + 128 * (c + 1)],
                                identity[:])
        if c % 2 == 0:
            nc.vector.tensor_copy(hsb[:, 512 * c:512 * (c + 1)], tH[:])
        else:
            nc.scalar.activation(hsb[:, 512 * c:512 * (c + 1)], tH[:],
                                 mybir.ActivationFunctionType.Identity)

    # ---- Q^T [j,d] fp32r ----
    tQ = tp_pool.tile([128, 512], F32, tag="tp", name=f"tQ{b}")
    for i in range(NCH):
        nc.tensor.transpose(tQ[:, 128 * i:128 * (i + 1)],
                            qt[:, J * i:J * (i + 1)], identity[:])
    QT = mid.tile([128, 512], F32R, tag="qT", bufs=2, name=f"qT{b}")
    nc.scalar.activation(QT[:], tQ[:],
                         mybir.ActivationFunctionType.Identity)

    # ---- q2c = (1/csum) * sum_c e_c[c].T @ H[c]  -> [j,d] fp32r ----
    psq = mm_pool.tile([128, 512], F32, tag="mm")
    for c in range(NTC):
        nc.tensor.matmul(psq[:], ec_sb[:, 128 * c:128 * (c + 1)],
                         hsb[:, 512 * c:512 * (c + 1)],
                         start=(c == 0), stop=(c == NTC - 1))
    q2c = mid.tile([128, 512], F32R, tag="q2c", bufs=2, name=f"q2c{b}")
    nc.vector.tensor_scalar_mul(q2c[:], psq[:], rc[:])

    # ---- A^T (fp32r) + H*A; A copies feed Pool early ----
    Am = outp.tile([128, NCH * T], F32, tag="am", bufs=2, name=f"am{b}")
    Ham = outp.tile([128, NCH * T], F32, tag="ham", bufs=2, name=f"ham{b}")
    for m in range(NCH):
        for h in range(2):
            sl = slice(512 * h, 512 * (h + 1))
            psA = mm_pool.tile([128, 512], F32, tag="mm", name=f"psA{b}_{m}{h}")
            nc.tensor.matmul(psA[:], QT[:, 128 * m:128 * (m + 1)], SqT[:, sl],
                             start=True, stop=True)
            nc.any.tensor_copy(Am[:, T * m + 512 * h:T * m + 512 * (h + 1)],
                               psA[:])
        eng = nc.gpsimd if m < 3 else nc.vector
        eng.tensor_mul(Ham[:, T * m:T * (m + 1)], ct[:, T * m:T * (m + 1)],
                       Am[:, T * m:T * (m + 1)])

    # ---- Bmat^T (fp32r), H*B ----
    Hbm = outp.tile([128, NCH * T], F32, tag="hbm", bufs=2, name=f"hbm{b}")
    for m in range(NCH):
        for h in range(2):
            sl = slice(512 * h, 512 * (h + 1))
            psB = mm_pool.tile([128, 512], F32, tag="mm", name=f"psB{b}_{m}{h}")
            nc.tensor.matmul(psB[:], q2c[:, 128 * m:128 * (m + 1)], SqT[:, sl],
                             start=True, stop=True)
            nc.vector.tensor_mul(Hbm[:, T * m + 512 * h:T * m + 512 * (h + 1)],
                                 ct[:, T * m + 512 * h:T * m + 512 * (h + 1)],
                                 psB[:])

    # ---- output stores (ACT queue): A, HA, HB blocks ----
    nc.scalar.dma_start(
        out[b, D:2 * D, :].rearrange("(c p) t -> p c t", c=NCH),
        Am[:].rearrange("p (c t) -> p c t", c=NCH))
    nc.scalar.dma_start(
        out[b, 2 * D:3 * D, :].rearrange("(c p) t -> p c t", c=NCH),
        Ham[:].rearrange("p (c t) -> p c t", c=NCH))
    nc.scalar.dma_start(
        out[b, 3 * D:4 * D, :].rearrange("(c p) t -> p c t", c=NCH),
        Hbm[:].rearrange("p (c t) -> p c t", c=NCH))


def _build():
    nc = bacc.Bacc("TRN2", target_bir_lowering=False, debug=False,
                   num_devices=NCORES)
    C = nc.dram_tensor("C", [BPC, D, T], F32, kind="ExternalInput").ap()
    Q = nc.dram_tensor("Q", [BPC, D, J], F32, kind="ExternalInput").ap()
    mask_C = nc.dram_tensor("mask_C", [BPC, T], I32, kind="ExternalInput").ap()
    mask_Q = nc.dram_tensor("mask_Q", [BPC, J], I32, kind="ExternalInput").ap()
    weight = nc.dram_tensor("weight", [3 * D], F32, kind="ExternalInput").ap()
    out = nc.dram_tensor("out", [BPC, 4 * D, T], F32,
                         kind="ExternalOutput").ap()

    with tile.TileContext(nc) as tc:
        import contextlib
        with contextlib.ExitStack() as ctx:
            const = ctx.enter_context(tc.tile_pool(name="const", bufs=1))
            cin = ctx.enter_context(tc.tile_pool(name="cin", bufs=2))
            qin = ctx.enter_context(tc.tile_pool(name="qin", bufs=2))
            mid = ctx.enter_context(tc.tile_pool(name="mid", bufs=2))
            outp = ctx.enter_context(tc.tile_pool(name="outp", bufs=2))
            sm = ctx.enter_context(tc.tile_pool(name="sm", bufs=4))
            psS_pool = ctx.enter_context(
                tc.tile_pool(name="psS", bufs=2, space="PSUM"))
            tp_pool = ctx.enter_context(
                tc.tile_pool(name="tp", bufs=3, space="PSUM"))
            mm_pool = ctx.enter_context(
                tc.tile_pool(name="mm", bufs=3, space="PSUM"))

            # ---- constants ----
            identity = const.tile([128, 128], F32, tag="identity")
            make_identity(nc, identity[:])
            ones128 = const.tile([128, 128], F32, tag="ones128")
            nc.gpsimd.memset(ones128[:], 1.0)
            onescol = const.tile([128, 8], F32R, tag="onescol")
            nc.vector.tensor_copy(onescol[:], ones128[:, 0:8])
            # weight -> [128, 12]: cols g*4+c hold weight[g*512 + c*128 + p]
            w_col = const.tile([128, 12], F32, tag="w_col")
            nc.sync.dma_start(
                w_col[:], weight.rearrange("(g c p) -> p (g c)", p=128, c=4))
            # W_H[i]: w_h chunk broadcast along free dim (rank-1 weights)
            W_H = []
            for i in range(NCH):
                t = const.tile([128, 128], F32R, tag=f"W_H{i}")
                nc.vector.tensor_scalar_mul(t[:], ones128[:], w_col[:, i:i + 1])
                W_H.append(t)
            # mask_C -> [128, BPC*8] fp32: col 8b+c holds mask_C[b, 128c+p]
            mci = const.tile([128, BPC * NTC], I32, tag="mci")
            nc.sync.dma_start(mci[:],
                              mask_C.rearrange("b (c p) -> p (b c)", p=128))
            mcolf = const.tile([128, BPC * NTC], F32, tag="mcolf")
            nc.vector.tensor_copy(mcolf[:], mci[:])
            # mask_Q -> [128, BPC] fp32
            mqi = const.tile([128, BPC], I32, tag="mqi")
            nc.sync.dma_start(mqi[:], mask_Q.rearrange("b j -> j b"))
            mqf_all = const.tile([128, BPC], F32, tag="mqf")
            nc.vector.tensor_copy(mqf_all[:], mqi[:])
            mhat_neg = const.tile([128, 1], F32, tag="mhat")
            nc.gpsimd.memset(mhat_neg[:], -MHAT)

            consts = (identity, ones128, onescol, w_col, W_H, mqf_all, mcolf,
                      mhat_neg)
            pools = (cin, qin, mid, outp, psS_pool, tp_pool, mm_pool, sm)
            for b in range(BPC):
                _emit_batch(nc, tc, pools, consts, (C, Q, out), b)

    nc.compile()
    return nc


_NC_CACHE = None


def _get_nc():
    global _NC_CACHE
    if _NC_CACHE is None:
        _NC_CACHE = _build()
    return _NC_CACHE


def kernel(C, Q, mask_C, mask_Q, weight):
    nc = _get_nc()
    C = np.ascontiguousarray(C, dtype=np.float32)
    Q = np.ascontiguousarray(Q, dtype=np.float32)
    mask_C = np.ascontiguousarray(mask_C, dtype=np.int32)
    mask_Q = np.ascontiguousarray(mask_Q, dtype=np.int32)
    weight = np.ascontiguousarray(weight, dtype=np.float32)
    in_maps = []
    for c in range(NCORES):
        sl = slice(BPC * c, BPC * (c + 1))
        in_maps.append({
            "C": C[sl], "Q": Q[sl], "mask_C": mask_C[sl],
            "mask_Q": mask_Q[sl], "weight": weight,
        })
    res = bass_utils.run_bass_kernel_spmd(nc, in_maps,
                                          core_ids=list(range(NCORES)))
    return np.concatenate([res.results[c]["out"] for c in range(NCORES)],
                          axis=0)
